# revision 12
# baseline (speedup 1.0000x reference)
"""Multi-head attention (B=2, S=2048, D=1024, H=16) on 8 trn2 NeuronCores.

Sharding: core c handles batch b=c//4, head-group hg=c%4 (heads hg*4..hg*4+3).
Per core:
  - projections: Q^T,K^T [256,S] fp32r (lhsT=W col-shard, rhs=host-transposed
    q^T/k^T), V [S,4*65] bf16 with a fused ones-column per head
  - scores computed TRANSPOSED (lhsT=K^T_h, rhs=Q^T_h) -> exp (ACT, scale=1/8)
    -> P'^T bf16; attn@V = matmul(lhsT=[V_h|1], rhs=P'^T) so no P transpose is
    needed for the value product, and the ones-column yields softmax row-sums
  - reciprocal done in transposed [128,4] layout (DVE divide is ~8 cyc/elem —
    layout must put only a few elems per lane)
  - P'^T tiles PE-transposed back to natural [Sq,Sk]; normalization fused into
    the PSUM->SBUF evict (tensor_scalar * recipT per-partition); 1 MiB DMAs out
  - fc: AllToAll within each 4-core batch group redistributes head-chunks to
    row-blocks, then full-channel [S/4,1024] @ W_o + b_o exactly (no reduce)
Host: transposes q/k/v per batch, shards weights, assembles attn + out.
"""

import numpy as np

import concourse.bacc as bacc
import concourse.bass as bass
import concourse.mybir as mybir
import concourse.tile as tile
from concourse.masks import make_identity

F32 = mybir.dt.float32
F32R = mybir.dt.float32r
BF16 = mybir.dt.bfloat16
AF = mybir.ActivationFunctionType
ALU = mybir.AluOpType

B = 2
D = 1024
H = 16
DK = 64
NCORES = 8
GROUP = 4  # cores per batch
HPC = 4  # heads per core
CH = HPC * DK  # 256 per-core channels
P = 128
KD = D // P  # 8 k-tiles over D
NBLK = 512  # Sq block width
SCALE = 1.0 / 8.0  # 1/sqrt(DK)


def build(S=2048):
    nc = bacc.Bacc("TRN2")
    ST = S // P  # Sq/Sk 128-tiles per head
    NB = S // NBLK  # Sq blocks
    SBG = S // GROUP  # rows per core after A2A

    qT = nc.declare_dram_parameter("qT", [D, S], F32R, isOutput=False)
    kT = nc.declare_dram_parameter("kT", [D, S], F32R, isOutput=False)
    vT = nc.declare_dram_parameter("vT", [D, S], F32R, isOutput=False)
    Wq = nc.declare_dram_parameter("Wq", [D, CH], F32R, isOutput=False)
    Wk = nc.declare_dram_parameter("Wk", [D, CH], F32R, isOutput=False)
    Wv = nc.declare_dram_parameter("Wv", [D, CH], F32R, isOutput=False)
    bq = nc.declare_dram_parameter("bq", [1, CH], F32R, isOutput=False)
    bk = nc.declare_dram_parameter("bk", [1, CH], F32R, isOutput=False)
    bv = nc.declare_dram_parameter("bv", [1, CH], F32R, isOutput=False)
    Wo = nc.declare_dram_parameter("Wo", [CH, D], F32R, isOutput=False)
    bo = nc.declare_dram_parameter("bo", [1, D], F32R, isOutput=False)
    attn_o = nc.declare_dram_parameter("attn", [HPC, S, S], F32, isOutput=True)
    y_o = nc.declare_dram_parameter("y", [SBG, D], F32, isOutput=True)

    with tile.TileContext(nc) as tc:
        with (
            tc.tile_pool(name="cpool", bufs=1) as cpool,
            tc.tile_pool(name="vpool", bufs=ST) as vpool,
            tc.tile_pool(name="otpool", bufs=2) as otpool,
            tc.tile_pool(name="ps_sc", bufs=2, space="PSUM") as ps_sc,
            tc.tile_pool(name="ps_ov", bufs=2, space="PSUM") as ps_ov,
            tc.tile_pool(name="ps_t", bufs=2, space="PSUM") as ps_t,
            tc.tile_pool(name="ps_misc", bufs=2, space="PSUM") as ps_misc,
        ):
            # ---- constants ----
            ident_b = cpool.tile([P, P], BF16, name="ident_b")
            make_identity(nc, ident_b)
            ident_f = cpool.tile([P, P], F32, name="ident_f")
            make_identity(nc, ident_f)
            ones_f = cpool.tile([1, NBLK], F32, name="ones_f")
            nc.vector.memset(ones_f[:], 1.0)
            ones_r = cpool.tile([1, NBLK], F32R, name="ones_r")
            nc.scalar.activation(ones_r[:], ones_f[:], AF.Copy)
            bq_sb = cpool.tile([1, CH], F32R, name="bq_sb")
            bk_sb = cpool.tile([1, CH], F32R, name="bk_sb")
            bv_sb = cpool.tile([1, CH], F32R, name="bv_sb")
            bo_sb = cpool.tile([1, D], F32R, name="bo_sb")
            nc.sync.dma_start(out=bq_sb[:], in_=bq[:, :])
            nc.sync.dma_start(out=bk_sb[:], in_=bk[:, :])
            nc.sync.dma_start(out=bv_sb[:], in_=bv[:, :])
            nc.sync.dma_start(out=bo_sb[:], in_=bo[:, :])

            # V stored bf16 interleaved [Sk, 4 heads x (64 ch | ones col)]
            v_tiles = [
                vpool.tile([P, HPC * (DK + 1)], BF16, tag="v", name=f"v_{i}")
                for i in range(ST)
            ]
            for t in v_tiles:
                nc.vector.memset(t[:], 1.0)

            # out^T accumulators: 2 tiles [128, S] f32r (4 heads x 64 ch)
            outT = [
                otpool.tile([P, S], F32R, tag="outT", name=f"outT_{i}")
                for i in range(2)
            ]

            # ---- projections ----
            qkt_cm = tc.tile_pool(name="qkt", bufs=2)
            qktpool = qkt_cm.__enter__()
            with (
                tc.tile_pool(name="wpool", bufs=3 * KD) as wpool,
                tc.tile_pool(name="vstream", bufs=KD) as vstream,
                tc.tile_pool(name="stream", bufs=6) as stream,
            ):
                w_tiles = {}
                for nm, W in (("q", Wq), ("k", Wk), ("v", Wv)):
                    for kk in range(KD):
                        t = wpool.tile([P, CH], F32R, tag="w", name=f"w{nm}{kk}")
                        nc.sync.dma_start(out=t[:], in_=W[kk * P : (kk + 1) * P, :])
                        w_tiles[nm, kk] = t

                QT = [
                    qktpool.tile([P, S], F32R, tag="qkt", name=f"QT_{m}")
                    for m in range(2)
                ]
                KTt = [
                    qktpool.tile([P, S], F32R, tag="qkt2", name=f"KT_{m}")
                    for m in range(2)
                ]

                # Q^T / K^T: psum[ch m, Sq n] = bias + sum_k Wx[k,m]^T @ xT[k,n]
                for nm, xT, dst, bias in (
                    ("q", qT, QT, bq_sb),
                    ("k", kT, KTt, bk_sb),
                ):
                    for n in range(NB):
                        psq = [
                            ps_sc.tile([P, NBLK], F32, tag="sc", name=f"ps{nm}{n}{m}")
                            for m in range(2)
                        ]
                        for m in range(2):
                            nc.tensor.matmul(
                                psq[m][:],
                                bias[:, m * P : (m + 1) * P],
                                ones_r[:],
                                start=True,
                                stop=False,
                            )
                        for kk in range(KD):
                            xc = stream.tile(
                                [P, NBLK], F32R, tag="xs", name=f"{nm}s{n}{kk}"
                            )
                            nc.sync.dma_start(
                                out=xc[:],
                                in_=xT[
                                    kk * P : (kk + 1) * P, n * NBLK : (n + 1) * NBLK
                                ],
                            )
                            for m in range(2):
                                nc.tensor.matmul(
                                    psq[m][:],
                                    w_tiles[nm, kk][:, m * P : (m + 1) * P],
                                    xc[:],
                                    start=False,
                                    stop=(kk == KD - 1),
                                )
                        for m in range(2):
                            nc.scalar.activation(
                                dst[m][:, n * NBLK : (n + 1) * NBLK],
                                psq[m][:],
                                AF.Copy,
                            )

                # V: psum[s tile, ch] = ones^T @ bv + sum_k vT[k,s]^T @ Wv[k]
                vt_full = [
                    vstream.tile([P, S], F32R, tag="vt", name=f"vt{kk}")
                    for kk in range(KD)
                ]
                for kk in range(KD):
                    nc.sync.dma_start(
                        out=vt_full[kk][:], in_=vT[kk * P : (kk + 1) * P, :]
                    )
                for s in range(ST):
                    psv = ps_ov.tile([P, CH], F32, tag="ov", name=f"psv{s}")
                    nc.tensor.matmul(
                        psv[:], ones_r[:, :P], bv_sb[:], start=True, stop=False
                    )
                    for kk in range(KD):
                        nc.tensor.matmul(
                            psv[:],
                            vt_full[kk][:, s * P : (s + 1) * P],
                            w_tiles["v", kk][:],
                            start=False,
                            stop=(kk == KD - 1),
                        )
                    nc.scalar.activation(
                        v_tiles[s][:].rearrange("p (h c) -> p h c", h=HPC)[
                            :, :, 0:DK
                        ],
                        psv[:].rearrange("p (h c) -> p h c", h=HPC),
                        AF.Copy,
                    )

            # ---- attention ----
            with (
                tc.tile_pool(name="ptpool", bufs=int(1.5 * ST)) as ptpool,
                tc.tile_pool(name="pnat", bufs=3) as pnat,
                tc.tile_pool(name="smalls", bufs=3) as smalls,
            ):
                for h in range(HPC):
                    ti, po = h // 2, (h % 2) * DK  # outT tile index / partition off
                    kth = KTt[h // 2]
                    qth = QT[h // 2]
                    vcol = h * (DK + 1)
                    for n in range(NB):
                        nsl = slice(n * NBLK, (n + 1) * NBLK)
                        # scores^T + exp -> P'^T bf16 tiles
                        pts = []
                        for kk in range(ST):
                            ps = ps_sc.tile(
                                [P, NBLK], F32, tag="sc", name=f"sc{h}_{n}_{kk}"
                            )
                            nc.tensor.matmul(
                                ps[:],
                                kth[po : po + DK, kk * P : (kk + 1) * P],
                                qth[po : po + DK, nsl],
                                start=True,
                                stop=True,
                            )
                            pt = ptpool.tile(
                                [P, NBLK], BF16, tag="pt", name=f"pt{h}_{n}_{kk}"
                            )
                            nc.scalar.activation(pt[:], ps[:], AF.Exp, scale=SCALE)
                            pts.append(pt)
                        # attn @ [V|1]: accumulate over Sk tiles
                        ps_o = ps_ov.tile(
                            [DK + 1, NBLK], F32, tag="ov", name=f"ov{h}_{n}"
                        )
                        for kk in range(ST):
                            nc.tensor.matmul(
                                ps_o[:],
                                v_tiles[kk][:, vcol : vcol + DK + 1],
                                pts[kk][:],
                                start=(kk == 0),
                                stop=(kk == ST - 1),
                            )
                        # sums row -> transposed reciprocal [128, 4]
                        sums_sb = smalls.tile(
                            [1, NBLK], F32, tag="sums", name=f"sums{h}_{n}"
                        )
                        nc.scalar.activation(sums_sb[:], ps_o[DK : DK + 1, :], AF.Copy)
                        ps_rt = ps_misc.tile(
                            [P, 4], F32, tag="misc", name=f"rt{h}_{n}"
                        )
                        for c in range(4):
                            nc.tensor.transpose(
                                ps_rt[:, c : c + 1],
                                sums_sb[0:1, c * P : (c + 1) * P],
                                ident_f[0:1, 0:1],
                            )
                        recipT = smalls.tile([P, 4], F32, tag="rT", name=f"rT{h}_{n}")
                        nc.vector.reciprocal(recipT[:], ps_rt[:, 0:4])
                        # row-layout recip for out^T normalization
                        ps_rr = ps_misc.tile(
                            [1, NBLK], F32, tag="misc", name=f"rr{h}_{n}"
                        )
                        for c in range(4):
                            nc.tensor.transpose(
                                ps_rr[0:1, c * P : (c + 1) * P],
                                recipT[:, c : c + 1],
                                ident_f[:],
                            )
                        rr_sb = smalls.tile(
                            [1, NBLK], F32R, tag="rr", name=f"rr{h}_{n}"
                        )
                        nc.scalar.activation(rr_sb[:], ps_rr[:], AF.Copy)
                        ps_bc = ps_misc.tile(
                            [DK, NBLK], F32, tag="misc", name=f"bc{h}_{n}"
                        )
                        nc.tensor.matmul(
                            ps_bc[:],
                            ones_r[:, :DK],
                            rr_sb[:],
                            start=True,
                            stop=True,
                        )
                        bc_sb = smalls.tile(
                            [DK, NBLK], F32, tag="bc", name=f"bcs{h}_{n}"
                        )
                        nc.scalar.activation(bc_sb[:], ps_bc[:], AF.Copy)
                        nc.vector.tensor_tensor(
                            outT[ti][po : po + DK, nsl],
                            ps_o[0:DK, :],
                            bc_sb[:],
                            ALU.mult,
                        )
                        # transpose P'^T back to natural + normalize + DMA out
                        for c in range(4):
                            st = n * 4 + c
                            pn = pnat.tile([P, S], F32, tag="pn", name=f"pn{h}_{st}")
                            for g in range(ST // 4):
                                ps_tt = ps_t.tile(
                                    [P, NBLK], BF16, tag="t", name=f"tt{h}_{st}_{g}"
                                )
                                for j in range(4):
                                    kk = g * 4 + j
                                    nc.tensor.transpose(
                                        ps_tt[:, j * P : (j + 1) * P],
                                        pts[kk][:, c * P : (c + 1) * P],
                                        ident_b[:],
                                    )
                                nc.vector.tensor_scalar_mul(
                                    pn[:, g * NBLK : (g + 1) * NBLK],
                                    ps_tt[:],
                                    recipT[:, c : c + 1],
                                )
                            nc.sync.dma_start(
                                out=attn_o[h, st * P : (st + 1) * P, :], in_=pn[:]
                            )

            qkt_cm.__exit__(None, None, None)

            # ---- fc (partial over own 256 channels) + ReduceScatter ----
            # Each core computes y_partial[S, D] = outT_own^T @ Wo[own rows] +
            # b_o/4, then ReduceScatter(add) over the 4-core batch group gives
            # each core its summed row-block [S/4, D]. Host passes b_o/GROUP.
            with (
                tc.tile_pool(name="dram", bufs=1, space="DRAM") as dram,
                tc.tile_pool(name="wopool", bufs=2) as wopool,
                tc.tile_pool(name="ypool", bufs=3) as ypool,
            ):
                rs_in = dram.tile([S, D], F32, name="rs_in")
                rs_out = dram.tile([SBG, D], F32, name="rs_out")
                wo_tiles = []
                for kk in range(2):
                    t = wopool.tile([P, D], F32R, tag="wo", name=f"wo{kk}")
                    nc.sync.dma_start(out=t[:], in_=Wo[kk * P : (kk + 1) * P, :])
                    wo_tiles.append(t)
                for mt in range(ST):
                    y_sb = ypool.tile([P, D], F32, tag="y", name=f"y{mt}")
                    for nchunk in range(2):
                        psy = ps_sc.tile(
                            [P, NBLK], F32, tag="sc", name=f"psy{mt}{nchunk}"
                        )
                        nc.tensor.matmul(
                            psy[:],
                            ones_r[:, :P],
                            bo_sb[:, nchunk * NBLK : (nchunk + 1) * NBLK],
                            start=True,
                            stop=False,
                        )
                        for kk in range(2):
                            nc.tensor.matmul(
                                psy[:],
                                outT[kk][:, mt * P : (mt + 1) * P],
                                wo_tiles[kk][:, nchunk * NBLK : (nchunk + 1) * NBLK],
                                start=False,
                                stop=(kk == 1),
                            )
                        nc.scalar.activation(
                            y_sb[:, nchunk * NBLK : (nchunk + 1) * NBLK],
                            psy[:],
                            AF.Copy,
                        )
                    nc.sync.dma_start(
                        out=rs_in[mt * P : (mt + 1) * P, :], in_=y_sb[:]
                    )
                nc.gpsimd.collective_compute(
                    "ReduceScatter",
                    ALU.add,
                    replica_groups=[[0, 1, 2, 3], [4, 5, 6, 7]],
                    ins=[rs_in.opt()],
                    outs=[rs_out.opt()],
                )
                nc.sync.dma_start(out=y_o[:, :], in_=rs_out[:])

    return nc


_NC_CACHE = {}


def _get_nc(S):
    if S not in _NC_CACHE:
        _NC_CACHE[S] = build(S)
        _NC_CACHE[S].finalize()
    return _NC_CACHE[S]


def make_in_maps(q, k, v, W_q, b_q, W_k, b_k, W_v, b_v, W_o, b_o):
    """Build the 8 per-core input dicts from full (numpy f32) inputs."""
    q, k, v = (np.asarray(x, np.float32) for x in (q, k, v))
    W_q, W_k, W_v, W_o = (np.asarray(x, np.float32) for x in (W_q, W_k, W_v, W_o))
    b_q, b_k, b_v, b_o = (np.asarray(x, np.float32) for x in (b_q, b_k, b_v, b_o))
    qT = [np.ascontiguousarray(q[b].T) for b in range(B)]
    kTv = [np.ascontiguousarray(k[b].T) for b in range(B)]
    vTv = [np.ascontiguousarray(v[b].T) for b in range(B)]
    in_maps = []
    for c in range(NCORES):
        b, hg = c // GROUP, c % GROUP
        cs = slice(hg * CH, (hg + 1) * CH)
        in_maps.append(
            {
                "qT": qT[b],
                "kT": kTv[b],
                "vT": vTv[b],
                "Wq": np.ascontiguousarray(W_q[:, cs]),
                "Wk": np.ascontiguousarray(W_k[:, cs]),
                "Wv": np.ascontiguousarray(W_v[:, cs]),
                "bq": np.ascontiguousarray(b_q[cs])[None, :],
                "bk": np.ascontiguousarray(b_k[cs])[None, :],
                "bv": np.ascontiguousarray(b_v[cs])[None, :],
                "Wo": np.ascontiguousarray(W_o[cs, :]),
                "bo": (b_o / GROUP)[None, :],
            }
        )
    return in_maps


def assemble(results, S):
    """Gather per-core outputs into full (out, attn)."""
    SBG = S // GROUP
    attn = np.empty((B, H, S, S), np.float32)
    out = np.empty((B, S, D), np.float32)
    for c in range(NCORES):
        b, hg = c // GROUP, c % GROUP
        attn[b, hg * HPC : (hg + 1) * HPC] = results[c]["attn"]
        out[b, hg * SBG : (hg + 1) * SBG] = results[c]["y"]
    return out, attn


def kernel(q, k, v, W_q, b_q, W_k, b_k, W_v, b_v, W_o, b_o):
    from concourse import bass2jax

    S = int(np.asarray(q).shape[1])
    nc = _get_nc(S)
    in_maps = make_in_maps(q, k, v, W_q, b_q, W_k, b_k, W_v, b_v, W_o, b_o)
    results = bass2jax.run_bass_via_pjrt(nc, in_maps, n_cores=NCORES)
    return assemble(results, S)


# revision 14
# speedup vs baseline: 1.2085x; 1.2085x over previous
"""Multi-head attention (B=2, S=2048, D=1024, H=16) on 8 trn2 NeuronCores.

Sharding: core c handles batch b=c//4, head-group hg=c%4 (heads hg*4..hg*4+3).
Per core:
  - projections: Q^T,K^T [256,S] fp32r (lhsT=W col-shard, rhs=host-transposed
    q^T/k^T), V [S,4*65] bf16 with a fused ones-column per head
  - scores computed TRANSPOSED (lhsT=K^T_h, rhs=Q^T_h) -> exp (ACT, scale=1/8)
    -> P'^T bf16; attn@V = matmul(lhsT=[V_h|1], rhs=P'^T) so no P transpose is
    needed for the value product, and the ones-column yields softmax row-sums
  - reciprocal done in transposed [128,4] layout (DVE divide is ~8 cyc/elem —
    layout must put only a few elems per lane)
  - P'^T tiles PE-transposed back to natural [Sq,Sk]; normalization fused into
    the PSUM->SBUF evict (tensor_scalar * recipT per-partition); 1 MiB DMAs out
  - fc: AllToAll within each 4-core batch group redistributes head-chunks to
    row-blocks, then full-channel [S/4,1024] @ W_o + b_o exactly (no reduce)
Host: transposes q/k/v per batch, shards weights, assembles attn + out.
"""

import numpy as np

import concourse.bacc as bacc
import concourse.bass as bass
import concourse.mybir as mybir
import concourse.tile as tile
from concourse.masks import make_identity

F32 = mybir.dt.float32
F32R = mybir.dt.float32r
BF16 = mybir.dt.bfloat16
AF = mybir.ActivationFunctionType
ALU = mybir.AluOpType

B = 2
D = 1024
H = 16
DK = 64
NCORES = 8
GROUP = 4  # cores per batch
HPC = 4  # heads per core
CH = HPC * DK  # 256 per-core channels
P = 128
KD = D // P  # 8 k-tiles over D
NBLK = 512  # Sq block width
SCALE = 1.0 / 8.0  # 1/sqrt(DK)


def build(S=2048):
    nc = bacc.Bacc("TRN2")
    ST = S // P  # Sq/Sk 128-tiles per head
    NB = S // NBLK  # Sq blocks
    SBG = S // GROUP  # rows per core after A2A

    qT = nc.declare_dram_parameter("qT", [D, S], F32R, isOutput=False)
    kT = nc.declare_dram_parameter("kT", [D, S], F32R, isOutput=False)
    vT = nc.declare_dram_parameter("vT", [D, S], F32R, isOutput=False)
    Wq = nc.declare_dram_parameter("Wq", [D, CH], F32R, isOutput=False)
    Wk = nc.declare_dram_parameter("Wk", [D, CH], F32R, isOutput=False)
    Wv = nc.declare_dram_parameter("Wv", [D, CH], F32R, isOutput=False)
    bq = nc.declare_dram_parameter("bq", [1, CH], F32R, isOutput=False)
    bk = nc.declare_dram_parameter("bk", [1, CH], F32R, isOutput=False)
    bv = nc.declare_dram_parameter("bv", [1, CH], F32R, isOutput=False)
    Wo = nc.declare_dram_parameter("Wo", [CH, D], F32R, isOutput=False)
    bo = nc.declare_dram_parameter("bo", [1, D], F32R, isOutput=False)
    attn_o = nc.declare_dram_parameter("attn", [HPC, S, S], F32, isOutput=True)
    y_o = nc.declare_dram_parameter("y", [SBG, D], F32, isOutput=True)

    with tile.TileContext(nc) as tc:
        with (
            tc.tile_pool(name="cpool", bufs=1) as cpool,
            tc.tile_pool(name="vpool", bufs=ST) as vpool,
            tc.tile_pool(name="otpool", bufs=2) as otpool,
            tc.tile_pool(name="ps_sc", bufs=2, space="PSUM") as ps_sc,
            tc.tile_pool(name="ps_ov", bufs=2, space="PSUM") as ps_ov,
            tc.tile_pool(name="ps_t", bufs=3, space="PSUM") as ps_t,
            tc.tile_pool(name="ps_misc", bufs=1, space="PSUM") as ps_misc,
        ):
            # ---- constants ----
            ident_b = cpool.tile([P, P], BF16, name="ident_b")
            make_identity(nc, ident_b)
            ident_f = cpool.tile([P, P], F32, name="ident_f")
            make_identity(nc, ident_f)
            ones_f = cpool.tile([1, NBLK], F32, name="ones_f")
            nc.vector.memset(ones_f[:], 1.0)
            ones_r = cpool.tile([1, NBLK], F32R, name="ones_r")
            nc.scalar.activation(ones_r[:], ones_f[:], AF.Copy)
            bq_sb = cpool.tile([1, CH], F32R, name="bq_sb")
            bk_sb = cpool.tile([1, CH], F32R, name="bk_sb")
            bv_sb = cpool.tile([1, CH], F32R, name="bv_sb")
            bo_sb = cpool.tile([1, D], F32R, name="bo_sb")
            nc.sync.dma_start(out=bq_sb[:], in_=bq[:, :])
            nc.sync.dma_start(out=bk_sb[:], in_=bk[:, :])
            nc.sync.dma_start(out=bv_sb[:], in_=bv[:, :])
            nc.sync.dma_start(out=bo_sb[:], in_=bo[:, :])

            # V stored bf16 interleaved [Sk, 4 heads x (64 ch | ones col)]
            v_tiles = [
                vpool.tile([P, HPC * (DK + 1)], BF16, tag="v", name=f"v_{i}")
                for i in range(ST)
            ]
            for t in v_tiles:
                nc.vector.memset(t[:], 1.0)

            # out^T accumulators: 2 tiles [128, S] f32r (4 heads x 64 ch)
            outT = [
                otpool.tile([P, S], F32R, tag="outT", name=f"outT_{i}")
                for i in range(2)
            ]

            # ---- projections ----
            qkt_cm = tc.tile_pool(name="qkt", bufs=2)
            qktpool = qkt_cm.__enter__()
            with (
                tc.tile_pool(name="wpool", bufs=3 * KD) as wpool,
                tc.tile_pool(name="vstream", bufs=KD) as vstream,
                tc.tile_pool(name="stream", bufs=6) as stream,
            ):
                w_tiles = {}
                for nm, W in (("q", Wq), ("k", Wk), ("v", Wv)):
                    for kk in range(KD):
                        t = wpool.tile([P, CH], F32R, tag="w", name=f"w{nm}{kk}")
                        nc.sync.dma_start(out=t[:], in_=W[kk * P : (kk + 1) * P, :])
                        w_tiles[nm, kk] = t

                QT = [
                    qktpool.tile([P, S], F32R, tag="qkt", name=f"QT_{m}")
                    for m in range(2)
                ]
                KTt = [
                    qktpool.tile([P, S], F32R, tag="qkt2", name=f"KT_{m}")
                    for m in range(2)
                ]

                # Q^T / K^T: psum[ch m, Sq n] = bias + sum_k Wx[k,m]^T @ xT[k,n]
                for nm, xT, dst, bias in (
                    ("q", qT, QT, bq_sb),
                    ("k", kT, KTt, bk_sb),
                ):
                    for n in range(NB):
                        psq = [
                            ps_sc.tile([P, NBLK], F32, tag="sc", name=f"ps{nm}{n}{m}")
                            for m in range(2)
                        ]
                        for m in range(2):
                            nc.tensor.matmul(
                                psq[m][:],
                                bias[:, m * P : (m + 1) * P],
                                ones_r[:],
                                start=True,
                                stop=False,
                            )
                        for kk in range(KD):
                            xc = stream.tile(
                                [P, NBLK], F32R, tag="xs", name=f"{nm}s{n}{kk}"
                            )
                            nc.sync.dma_start(
                                out=xc[:],
                                in_=xT[
                                    kk * P : (kk + 1) * P, n * NBLK : (n + 1) * NBLK
                                ],
                            )
                            for m in range(2):
                                nc.tensor.matmul(
                                    psq[m][:],
                                    w_tiles[nm, kk][:, m * P : (m + 1) * P],
                                    xc[:],
                                    start=False,
                                    stop=(kk == KD - 1),
                                )
                        for m in range(2):
                            nc.scalar.activation(
                                dst[m][:, n * NBLK : (n + 1) * NBLK],
                                psq[m][:],
                                AF.Copy,
                            )

                # V: psum[s tile, ch] = ones^T @ bv + sum_k vT[k,s]^T @ Wv[k]
                vt_full = [
                    vstream.tile([P, S], F32R, tag="vt", name=f"vt{kk}")
                    for kk in range(KD)
                ]
                for kk in range(KD):
                    nc.sync.dma_start(
                        out=vt_full[kk][:], in_=vT[kk * P : (kk + 1) * P, :]
                    )
                for s in range(ST):
                    psv = ps_ov.tile([P, CH], F32, tag="ov", name=f"psv{s}")
                    nc.tensor.matmul(
                        psv[:], ones_r[:, :P], bv_sb[:], start=True, stop=False
                    )
                    for kk in range(KD):
                        nc.tensor.matmul(
                            psv[:],
                            vt_full[kk][:, s * P : (s + 1) * P],
                            w_tiles["v", kk][:],
                            start=False,
                            stop=(kk == KD - 1),
                        )
                    nc.scalar.activation(
                        v_tiles[s][:].rearrange("p (h c) -> p h c", h=HPC)[
                            :, :, 0:DK
                        ],
                        psv[:].rearrange("p (h c) -> p h c", h=HPC),
                        AF.Copy,
                    )

            # ---- attention (block-outer) + per-block fc/ReduceScatter ----
            with (
                tc.tile_pool(name="ptpool", bufs=2 * ST) as ptpool,
                tc.tile_pool(name="pnat", bufs=3) as pnat,
                tc.tile_pool(name="smalls", bufs=4) as smalls,
                tc.tile_pool(name="dram", bufs=NB, space="DRAM") as dram,
                tc.tile_pool(name="wopool", bufs=2) as wopool,
                tc.tile_pool(name="ypool", bufs=3) as ypool,
            ):
                rs_in = [
                    dram.tile([NBLK, D], F32, tag="rsin", name=f"rs_in{n}")
                    for n in range(NB)
                ]
                rs_out = [
                    dram.tile([NBLK // GROUP, D], F32, tag="rsout", name=f"rs_out{n}")
                    for n in range(NB)
                ]
                wo_tiles = []
                for kk in range(2):
                    t = wopool.tile([P, D], F32R, tag="wo", name=f"wo{kk}")
                    nc.sync.dma_start(out=t[:], in_=Wo[kk * P : (kk + 1) * P, :])
                    wo_tiles.append(t)

                for n in range(NB):
                    nsl = slice(n * NBLK, (n + 1) * NBLK)
                    for h in range(HPC):
                        ti, po = h // 2, (h % 2) * DK
                        kth = KTt[h // 2]
                        qth = QT[h // 2]
                        vcol = h * (DK + 1)
                        # scores^T + exp -> P'^T bf16 tiles
                        pts = []
                        for kk in range(ST):
                            ps = ps_sc.tile(
                                [P, NBLK], F32, tag="sc", name=f"sc{h}_{n}_{kk}"
                            )
                            nc.tensor.matmul(
                                ps[:],
                                kth[po : po + DK, kk * P : (kk + 1) * P],
                                qth[po : po + DK, nsl],
                                start=True,
                                stop=True,
                            )
                            pt = ptpool.tile(
                                [P, NBLK], BF16, tag="pt", name=f"pt{h}_{n}_{kk}"
                            )
                            nc.scalar.activation(pt[:], ps[:], AF.Exp, scale=SCALE)
                            pts.append(pt)
                        # attn @ [V|1]: accumulate over Sk tiles
                        ps_o = ps_ov.tile(
                            [DK + 1, NBLK], F32, tag="ov", name=f"ov{h}_{n}"
                        )
                        for kk in range(ST):
                            nc.tensor.matmul(
                                ps_o[:],
                                v_tiles[kk][:, vcol : vcol + DK + 1],
                                pts[kk][:],
                                start=(kk == 0),
                                stop=(kk == ST - 1),
                            )
                        # sums row -> transposed reciprocal [128, 4]
                        sums_sb = smalls.tile(
                            [1, NBLK], F32, tag="sums", name=f"sums{h}_{n}"
                        )
                        nc.vector.tensor_copy(sums_sb[:], ps_o[DK : DK + 1, :])
                        ps_rt = ps_misc.tile([P, 4], F32, tag="misc", name=f"rt{h}_{n}")
                        for c in range(4):
                            nc.tensor.transpose(
                                ps_rt[:, c : c + 1],
                                sums_sb[0:1, c * P : (c + 1) * P],
                                ident_f[0:1, 0:1],
                            )
                        recipT = smalls.tile([P, 4], F32, tag="rT", name=f"rT{h}_{n}")
                        nc.vector.reciprocal(recipT[:], ps_rt[:, 0:4])
                        # transpose P'^T back to natural + normalize + DMA out
                        for c in range(4):
                            st = n * 4 + c
                            pn = pnat.tile([P, S], F32, tag="pn", name=f"pn{h}_{st}")
                            for g in range(ST // 4):
                                ps_tt = ps_t.tile(
                                    [P, NBLK], BF16, tag="t", name=f"tt{h}_{st}_{g}"
                                )
                                for j in range(4):
                                    kk = g * 4 + j
                                    nc.tensor.transpose(
                                        ps_tt[:, j * P : (j + 1) * P],
                                        pts[kk][:, c * P : (c + 1) * P],
                                        ident_b[:],
                                    )
                                nc.vector.tensor_scalar_mul(
                                    pn[:, g * NBLK : (g + 1) * NBLK],
                                    ps_tt[:],
                                    recipT[:, c : c + 1],
                                )
                            nc.sync.dma_start(
                                out=attn_o[h, st * P : (st + 1) * P, :], in_=pn[:]
                            )
                        # row-layout recip -> broadcast -> normalized out^T
                        # (off the transpose critical path)
                        ps_rr = ps_misc.tile(
                            [1, NBLK], F32, tag="misc", name=f"rr{h}_{n}"
                        )
                        for c in range(4):
                            nc.tensor.transpose(
                                ps_rr[0:1, c * P : (c + 1) * P],
                                recipT[:, c : c + 1],
                                ident_f[:],
                            )
                        rr_sb = smalls.tile([1, NBLK], F32R, tag="rr", name=f"rr{h}_{n}")
                        nc.vector.tensor_copy(rr_sb[:], ps_rr[:])
                        ps_bc = ps_misc.tile(
                            [DK, NBLK], F32, tag="misc", name=f"bc{h}_{n}"
                        )
                        nc.tensor.matmul(
                            ps_bc[:], ones_r[:, :DK], rr_sb[:], start=True, stop=True
                        )
                        bc_sb = smalls.tile([DK, NBLK], F32, tag="bc", name=f"bcs{h}_{n}")
                        nc.vector.tensor_copy(bc_sb[:], ps_bc[:])
                        nc.vector.tensor_tensor(
                            outT[ti][po : po + DK, nsl],
                            ps_o[0:DK, :],
                            bc_sb[:],
                            ALU.mult,
                        )

                    # fc for this row block + ReduceScatter chunk
                    for c in range(4):
                        mt = n * 4 + c
                        y_sb = ypool.tile([P, D], F32, tag="y", name=f"y{mt}")
                        for nchunk in range(2):
                            psy = ps_sc.tile(
                                [P, NBLK], F32, tag="sc", name=f"psy{mt}{nchunk}"
                            )
                            nc.tensor.matmul(
                                psy[:],
                                ones_r[:, :P],
                                bo_sb[:, nchunk * NBLK : (nchunk + 1) * NBLK],
                                start=True,
                                stop=False,
                            )
                            for kk in range(2):
                                nc.tensor.matmul(
                                    psy[:],
                                    outT[kk][:, mt * P : (mt + 1) * P],
                                    wo_tiles[kk][:, nchunk * NBLK : (nchunk + 1) * NBLK],
                                    start=False,
                                    stop=(kk == 1),
                                )
                            nc.scalar.activation(
                                y_sb[:, nchunk * NBLK : (nchunk + 1) * NBLK],
                                psy[:],
                                AF.Copy,
                            )
                        nc.sync.dma_start(
                            out=rs_in[n][c * P : (c + 1) * P, :], in_=y_sb[:]
                        )
                    nc.gpsimd.collective_compute(
                        "ReduceScatter",
                        ALU.add,
                        replica_groups=[[0, 1, 2, 3], [4, 5, 6, 7]],
                        ins=[rs_in[n].opt()],
                        outs=[rs_out[n].opt()],
                    )
                    nbg = NBLK // GROUP
                    nc.sync.dma_start(
                        out=y_o[n * nbg : (n + 1) * nbg, :], in_=rs_out[n][:]
                    )

            qkt_cm.__exit__(None, None, None)

    return nc


_NC_CACHE = {}


def _get_nc(S):
    if S not in _NC_CACHE:
        _NC_CACHE[S] = build(S)
        _NC_CACHE[S].finalize()
    return _NC_CACHE[S]


def make_in_maps(q, k, v, W_q, b_q, W_k, b_k, W_v, b_v, W_o, b_o):
    """Build the 8 per-core input dicts from full (numpy f32) inputs."""
    q, k, v = (np.asarray(x, np.float32) for x in (q, k, v))
    W_q, W_k, W_v, W_o = (np.asarray(x, np.float32) for x in (W_q, W_k, W_v, W_o))
    b_q, b_k, b_v, b_o = (np.asarray(x, np.float32) for x in (b_q, b_k, b_v, b_o))
    qT = [np.ascontiguousarray(q[b].T) for b in range(B)]
    kTv = [np.ascontiguousarray(k[b].T) for b in range(B)]
    vTv = [np.ascontiguousarray(v[b].T) for b in range(B)]
    in_maps = []
    for c in range(NCORES):
        b, hg = c // GROUP, c % GROUP
        cs = slice(hg * CH, (hg + 1) * CH)
        in_maps.append(
            {
                "qT": qT[b],
                "kT": kTv[b],
                "vT": vTv[b],
                "Wq": np.ascontiguousarray(W_q[:, cs]),
                "Wk": np.ascontiguousarray(W_k[:, cs]),
                "Wv": np.ascontiguousarray(W_v[:, cs]),
                "bq": np.ascontiguousarray(b_q[cs])[None, :],
                "bk": np.ascontiguousarray(b_k[cs])[None, :],
                "bv": np.ascontiguousarray(b_v[cs])[None, :],
                "Wo": np.ascontiguousarray(W_o[cs, :]),
                "bo": (b_o / GROUP)[None, :],
            }
        )
    return in_maps


def assemble(results, S):
    """Gather per-core outputs into full (out, attn)."""
    NB = S // NBLK
    nbg = NBLK // GROUP
    attn = np.empty((B, H, S, S), np.float32)
    out = np.empty((B, S, D), np.float32)
    for c in range(NCORES):
        b, j = c // GROUP, c % GROUP
        attn[b, j * HPC : (j + 1) * HPC] = results[c]["attn"]
        y = results[c]["y"].reshape(NB, nbg, D)
        for n in range(NB):
            out[b, n * NBLK + j * nbg : n * NBLK + (j + 1) * nbg] = y[n]
    return out, attn


def kernel(q, k, v, W_q, b_q, W_k, b_k, W_v, b_v, W_o, b_o):
    from concourse import bass2jax

    S = int(np.asarray(q).shape[1])
    nc = _get_nc(S)
    in_maps = make_in_maps(q, k, v, W_q, b_q, W_k, b_k, W_v, b_v, W_o, b_o)
    results = bass2jax.run_bass_via_pjrt(nc, in_maps, n_cores=NCORES)
    return assemble(results, S)


# revision 16
# speedup vs baseline: 4.4303x; 3.6660x over previous
"""Multi-head attention (B=2, S=2048, D=1024, H=16) on 8 trn2 NeuronCores.

Sharding: core c handles batch b=c//4, head-group hg=c%4 (heads hg*4..hg*4+3).
Per core:
  - projections: Q^T,K^T [256,S] fp32r (lhsT=W col-shard, rhs=host-transposed
    q^T/k^T), V [S,4*65] bf16 with a fused ones-column per head
  - scores computed TRANSPOSED (lhsT=K^T_h, rhs=Q^T_h) -> exp (ACT, scale=1/8)
    -> P'^T bf16; attn@V = matmul(lhsT=[V_h|1], rhs=P'^T) so no P transpose is
    needed for the value product, and the ones-column yields softmax row-sums
  - reciprocal done in transposed [128,4] layout (DVE divide is ~8 cyc/elem —
    layout must put only a few elems per lane)
  - P'^T tiles PE-transposed back to natural [Sq,Sk]; normalization fused into
    the PSUM->SBUF evict (tensor_scalar * recipT per-partition); 1 MiB DMAs out
  - fc: AllToAll within each 4-core batch group redistributes head-chunks to
    row-blocks, then full-channel [S/4,1024] @ W_o + b_o exactly (no reduce)
Host: transposes q/k/v per batch, shards weights, assembles attn + out.
"""

import numpy as np

import concourse.bacc as bacc
import concourse.bass as bass
import concourse.mybir as mybir
import concourse.tile as tile
from concourse.masks import make_identity

F32 = mybir.dt.float32
F32R = mybir.dt.float32r
BF16 = mybir.dt.bfloat16
AF = mybir.ActivationFunctionType
ALU = mybir.AluOpType

B = 2
D = 1024
H = 16
DK = 64
NCORES = 8
GROUP = 4  # cores per batch
HPC = 4  # heads per core
CH = HPC * DK  # 256 per-core channels
P = 128
KD = D // P  # 8 k-tiles over D
NBLK = 512  # Sq block width
SCALE = 1.0 / 8.0  # 1/sqrt(DK)


def build(S=2048):
    nc = bacc.Bacc("TRN2")
    ST = S // P  # Sq/Sk 128-tiles per head
    NB = S // NBLK  # Sq blocks
    SBG = S // GROUP  # rows per core after A2A

    qT = nc.declare_dram_parameter("qT", [D, S], F32R, isOutput=False)
    kT = nc.declare_dram_parameter("kT", [D, S], F32R, isOutput=False)
    vT = nc.declare_dram_parameter("vT", [D, S], F32R, isOutput=False)
    Wq = nc.declare_dram_parameter("Wq", [D, CH], F32R, isOutput=False)
    Wk = nc.declare_dram_parameter("Wk", [D, CH], F32R, isOutput=False)
    Wv = nc.declare_dram_parameter("Wv", [D, CH], F32R, isOutput=False)
    bq = nc.declare_dram_parameter("bq", [1, CH], F32R, isOutput=False)
    bk = nc.declare_dram_parameter("bk", [1, CH], F32R, isOutput=False)
    bv = nc.declare_dram_parameter("bv", [1, CH], F32R, isOutput=False)
    Wo = nc.declare_dram_parameter("Wo", [CH, D], F32R, isOutput=False)
    bo = nc.declare_dram_parameter("bo", [1, D], F32R, isOutput=False)
    attn_o = nc.declare_dram_parameter("attn", [HPC, S, S], F32, isOutput=True)
    y_o = nc.declare_dram_parameter("y", [SBG, D], F32, isOutput=True)

    with tile.TileContext(nc) as tc:
        with (
            tc.tile_pool(name="cpool", bufs=1) as cpool,
            tc.tile_pool(name="vpool", bufs=ST) as vpool,
            tc.tile_pool(name="otpool", bufs=2) as otpool,
            tc.tile_pool(name="ps_sc", bufs=4, space="PSUM") as ps_sc,
            tc.tile_pool(name="ps_ov", bufs=2, space="PSUM") as ps_ov,
            tc.tile_pool(name="ps_misc", bufs=2, space="PSUM") as ps_misc,
        ):
            # ---- constants ----
            ident_f = cpool.tile([P, P], F32, name="ident_f")
            make_identity(nc, ident_f)
            ones_f = cpool.tile([1, NBLK], F32, name="ones_f")
            nc.vector.memset(ones_f[:], 1.0)
            ones_r = cpool.tile([1, NBLK], F32R, name="ones_r")
            nc.scalar.activation(ones_r[:], ones_f[:], AF.Copy)
            bq_sb = cpool.tile([1, CH], F32R, name="bq_sb")
            bk_sb = cpool.tile([1, CH], F32R, name="bk_sb")
            bv_sb = cpool.tile([1, CH], F32R, name="bv_sb")
            bo_sb = cpool.tile([1, D], F32R, name="bo_sb")
            nc.sync.dma_start(out=bq_sb[:], in_=bq[:, :])
            nc.sync.dma_start(out=bk_sb[:], in_=bk[:, :])
            nc.sync.dma_start(out=bv_sb[:], in_=bv[:, :])
            nc.sync.dma_start(out=bo_sb[:], in_=bo[:, :])

            # V stored bf16 interleaved [Sk, 4 heads x (64 ch | ones col)]
            v_tiles = [
                vpool.tile([P, HPC * (DK + 1)], BF16, tag="v", name=f"v_{i}")
                for i in range(ST)
            ]
            for t in v_tiles:
                nc.vector.memset(t[:], 1.0)

            # out^T accumulators: 2 tiles [128, S] f32r (4 heads x 64 ch)
            outT = [
                otpool.tile([P, S], F32R, tag="outT", name=f"outT_{i}")
                for i in range(2)
            ]

            # ---- projections ----
            qkt_cm = tc.tile_pool(name="qkt", bufs=2)
            qktpool = qkt_cm.__enter__()
            with (
                tc.tile_pool(name="wpool", bufs=3 * KD) as wpool,
                tc.tile_pool(name="vstream", bufs=KD) as vstream,
                tc.tile_pool(name="stream", bufs=6) as stream,
            ):
                w_tiles = {}
                for nm, W in (("q", Wq), ("k", Wk), ("v", Wv)):
                    for kk in range(KD):
                        t = wpool.tile([P, CH], F32R, tag="w", name=f"w{nm}{kk}")
                        nc.sync.dma_start(out=t[:], in_=W[kk * P : (kk + 1) * P, :])
                        w_tiles[nm, kk] = t

                QT = [
                    qktpool.tile([P, S], F32R, tag="qkt", name=f"QT_{m}")
                    for m in range(2)
                ]
                KTt = [
                    qktpool.tile([P, S], F32R, tag="qkt2", name=f"KT_{m}")
                    for m in range(2)
                ]

                # Q^T / K^T: psum[ch m, Sq n] = bias + sum_k Wx[k,m]^T @ xT[k,n]
                for nm, xT, dst, bias in (
                    ("q", qT, QT, bq_sb),
                    ("k", kT, KTt, bk_sb),
                ):
                    for n in range(NB):
                        psq = [
                            ps_sc.tile([P, NBLK], F32, tag="sc", name=f"ps{nm}{n}{m}")
                            for m in range(2)
                        ]
                        for m in range(2):
                            nc.tensor.matmul(
                                psq[m][:],
                                bias[:, m * P : (m + 1) * P],
                                ones_r[:],
                                start=True,
                                stop=False,
                            )
                        for kk in range(KD):
                            xc = stream.tile(
                                [P, NBLK], F32R, tag="xs", name=f"{nm}s{n}{kk}"
                            )
                            nc.sync.dma_start(
                                out=xc[:],
                                in_=xT[
                                    kk * P : (kk + 1) * P, n * NBLK : (n + 1) * NBLK
                                ],
                            )
                            for m in range(2):
                                nc.tensor.matmul(
                                    psq[m][:],
                                    w_tiles[nm, kk][:, m * P : (m + 1) * P],
                                    xc[:],
                                    start=False,
                                    stop=(kk == KD - 1),
                                )
                        for m in range(2):
                            nc.scalar.activation(
                                dst[m][:, n * NBLK : (n + 1) * NBLK],
                                psq[m][:],
                                AF.Copy,
                            )

                # V: psum[s tile, ch] = ones^T @ bv + sum_k vT[k,s]^T @ Wv[k]
                vt_full = [
                    vstream.tile([P, S], F32R, tag="vt", name=f"vt{kk}")
                    for kk in range(KD)
                ]
                for kk in range(KD):
                    nc.sync.dma_start(
                        out=vt_full[kk][:], in_=vT[kk * P : (kk + 1) * P, :]
                    )
                for s in range(ST):
                    psv = ps_ov.tile([P, CH], F32, tag="ov", name=f"psv{s}")
                    nc.tensor.matmul(
                        psv[:], ones_r[:, :P], bv_sb[:], start=True, stop=False
                    )
                    for kk in range(KD):
                        nc.tensor.matmul(
                            psv[:],
                            vt_full[kk][:, s * P : (s + 1) * P],
                            w_tiles["v", kk][:],
                            start=False,
                            stop=(kk == KD - 1),
                        )
                    nc.scalar.activation(
                        v_tiles[s][:].rearrange("p (h c) -> p h c", h=HPC)[
                            :, :, 0:DK
                        ],
                        psv[:].rearrange("p (h c) -> p h c", h=HPC),
                        AF.Copy,
                    )

            # ---- attention (block-outer) + per-block fc/ReduceScatter ----
            with (
                tc.tile_pool(name="ptpool", bufs=2 * ST) as ptpool,
                tc.tile_pool(name="pnat", bufs=3) as pnat,
                tc.tile_pool(name="smalls", bufs=4) as smalls,
                tc.tile_pool(name="dram", bufs=NB, space="DRAM") as dram,
                tc.tile_pool(name="wopool", bufs=2) as wopool,
                tc.tile_pool(name="ypool", bufs=3) as ypool,
            ):
                rs_in = [
                    dram.tile([NBLK, D], F32, tag="rsin", name=f"rs_in{n}")
                    for n in range(NB)
                ]
                rs_out = [
                    dram.tile([NBLK // GROUP, D], F32, tag="rsout", name=f"rs_out{n}")
                    for n in range(NB)
                ]
                wo_tiles = []
                for kk in range(2):
                    t = wopool.tile([P, D], F32R, tag="wo", name=f"wo{kk}")
                    nc.sync.dma_start(out=t[:], in_=Wo[kk * P : (kk + 1) * P, :])
                    wo_tiles.append(t)

                for n in range(NB):
                    nsl = slice(n * NBLK, (n + 1) * NBLK)
                    for h in range(HPC):
                        ti, po = h // 2, (h % 2) * DK
                        kth = KTt[h // 2]
                        qth = QT[h // 2]
                        vcol = h * (DK + 1)
                        # scores^T + exp -> P'^T bf16 tiles
                        pts = []
                        for kk in range(ST):
                            ps = ps_sc.tile(
                                [P, NBLK], F32, tag="sc", name=f"sc{h}_{n}_{kk}"
                            )
                            nc.tensor.matmul(
                                ps[:],
                                kth[po : po + DK, kk * P : (kk + 1) * P],
                                qth[po : po + DK, nsl],
                                start=True,
                                stop=True,
                            )
                            pt = ptpool.tile(
                                [P, NBLK], BF16, tag="pt", name=f"pt{h}_{n}_{kk}"
                            )
                            nc.scalar.activation(pt[:], ps[:], AF.Exp, scale=SCALE)
                            pts.append(pt)
                        # attn @ [V|1]: accumulate over Sk tiles
                        ps_o = ps_ov.tile(
                            [DK + 1, NBLK], F32, tag="ov", name=f"ov{h}_{n}"
                        )
                        for kk in range(ST):
                            nc.tensor.matmul(
                                ps_o[:],
                                v_tiles[kk][:, vcol : vcol + DK + 1],
                                pts[kk][:],
                                start=(kk == 0),
                                stop=(kk == ST - 1),
                            )
                        # sums row -> transposed reciprocal [128, 4]
                        sums_sb = smalls.tile(
                            [1, NBLK], F32, tag="sums", name=f"sums{h}_{n}"
                        )
                        nc.vector.tensor_copy(sums_sb[:], ps_o[DK : DK + 1, :])
                        ps_rt = ps_misc.tile([P, 4], F32, tag="misc", name=f"rt{h}_{n}")
                        for c in range(4):
                            nc.tensor.transpose(
                                ps_rt[:, c : c + 1],
                                sums_sb[0:1, c * P : (c + 1) * P],
                                ident_f[0:1, 0:1],
                            )
                        recipT = smalls.tile([P, 4], F32, tag="rT", name=f"rT{h}_{n}")
                        nc.vector.reciprocal(recipT[:], ps_rt[:, 0:4])
                        # natural-orientation scores + exp -> P_nat bf16,
                        # then per-partition normalize (DVE) -> f32 -> DMA out
                        for c in range(4):
                            st = n * 4 + c
                            praw = pnat.tile(
                                [P, S], BF16, tag="praw", name=f"praw{h}_{st}"
                            )
                            for g in range(ST // 4):
                                ps2 = ps_sc.tile(
                                    [P, NBLK], F32, tag="sc", name=f"sn{h}_{st}_{g}"
                                )
                                nc.tensor.matmul(
                                    ps2[:],
                                    qth[po : po + DK, st * P : (st + 1) * P],
                                    kth[po : po + DK, g * NBLK : (g + 1) * NBLK],
                                    start=True,
                                    stop=True,
                                )
                                nc.scalar.activation(
                                    praw[:, g * NBLK : (g + 1) * NBLK],
                                    ps2[:],
                                    AF.Exp,
                                    scale=SCALE,
                                )
                            pn = pnat.tile([P, S], F32, tag="pn", name=f"pn{h}_{st}")
                            nc.vector.tensor_scalar_mul(
                                pn[:], praw[:], recipT[:, c : c + 1]
                            )
                            nc.sync.dma_start(
                                out=attn_o[h, st * P : (st + 1) * P, :], in_=pn[:]
                            )
                        # row-layout recip -> broadcast -> normalized out^T
                        # (off the transpose critical path)
                        ps_rr = ps_misc.tile(
                            [1, NBLK], F32, tag="misc", name=f"rr{h}_{n}"
                        )
                        for c in range(4):
                            nc.tensor.transpose(
                                ps_rr[0:1, c * P : (c + 1) * P],
                                recipT[:, c : c + 1],
                                ident_f[:],
                            )
                        rr_sb = smalls.tile([1, NBLK], F32R, tag="rr", name=f"rr{h}_{n}")
                        nc.vector.tensor_copy(rr_sb[:], ps_rr[:])
                        ps_bc = ps_misc.tile(
                            [DK, NBLK], F32, tag="misc", name=f"bc{h}_{n}"
                        )
                        nc.tensor.matmul(
                            ps_bc[:], ones_r[:, :DK], rr_sb[:], start=True, stop=True
                        )
                        bc_sb = smalls.tile([DK, NBLK], F32, tag="bc", name=f"bcs{h}_{n}")
                        nc.vector.tensor_copy(bc_sb[:], ps_bc[:])
                        nc.vector.tensor_tensor(
                            outT[ti][po : po + DK, nsl],
                            ps_o[0:DK, :],
                            bc_sb[:],
                            ALU.mult,
                        )

                    # fc for this row block + ReduceScatter chunk
                    for c in range(4):
                        mt = n * 4 + c
                        y_sb = ypool.tile([P, D], F32, tag="y", name=f"y{mt}")
                        for nchunk in range(2):
                            psy = ps_sc.tile(
                                [P, NBLK], F32, tag="sc", name=f"psy{mt}{nchunk}"
                            )
                            nc.tensor.matmul(
                                psy[:],
                                ones_r[:, :P],
                                bo_sb[:, nchunk * NBLK : (nchunk + 1) * NBLK],
                                start=True,
                                stop=False,
                            )
                            for kk in range(2):
                                nc.tensor.matmul(
                                    psy[:],
                                    outT[kk][:, mt * P : (mt + 1) * P],
                                    wo_tiles[kk][:, nchunk * NBLK : (nchunk + 1) * NBLK],
                                    start=False,
                                    stop=(kk == 1),
                                )
                            nc.scalar.activation(
                                y_sb[:, nchunk * NBLK : (nchunk + 1) * NBLK],
                                psy[:],
                                AF.Copy,
                            )
                        nc.sync.dma_start(
                            out=rs_in[n][c * P : (c + 1) * P, :], in_=y_sb[:]
                        )
                    nc.gpsimd.collective_compute(
                        "ReduceScatter",
                        ALU.add,
                        replica_groups=[[0, 1, 2, 3], [4, 5, 6, 7]],
                        ins=[rs_in[n].opt()],
                        outs=[rs_out[n].opt()],
                    )
                    nbg = NBLK // GROUP
                    nc.sync.dma_start(
                        out=y_o[n * nbg : (n + 1) * nbg, :], in_=rs_out[n][:]
                    )

            qkt_cm.__exit__(None, None, None)

    return nc


_NC_CACHE = {}


def _get_nc(S):
    if S not in _NC_CACHE:
        _NC_CACHE[S] = build(S)
        _NC_CACHE[S].finalize()
    return _NC_CACHE[S]


def make_in_maps(q, k, v, W_q, b_q, W_k, b_k, W_v, b_v, W_o, b_o):
    """Build the 8 per-core input dicts from full (numpy f32) inputs."""
    q, k, v = (np.asarray(x, np.float32) for x in (q, k, v))
    W_q, W_k, W_v, W_o = (np.asarray(x, np.float32) for x in (W_q, W_k, W_v, W_o))
    b_q, b_k, b_v, b_o = (np.asarray(x, np.float32) for x in (b_q, b_k, b_v, b_o))
    qT = [np.ascontiguousarray(q[b].T) for b in range(B)]
    kTv = [np.ascontiguousarray(k[b].T) for b in range(B)]
    vTv = [np.ascontiguousarray(v[b].T) for b in range(B)]
    in_maps = []
    for c in range(NCORES):
        b, hg = c // GROUP, c % GROUP
        cs = slice(hg * CH, (hg + 1) * CH)
        in_maps.append(
            {
                "qT": qT[b],
                "kT": kTv[b],
                "vT": vTv[b],
                "Wq": np.ascontiguousarray(W_q[:, cs]),
                "Wk": np.ascontiguousarray(W_k[:, cs]),
                "Wv": np.ascontiguousarray(W_v[:, cs]),
                "bq": np.ascontiguousarray(b_q[cs])[None, :],
                "bk": np.ascontiguousarray(b_k[cs])[None, :],
                "bv": np.ascontiguousarray(b_v[cs])[None, :],
                "Wo": np.ascontiguousarray(W_o[cs, :]),
                "bo": (b_o / GROUP)[None, :],
            }
        )
    return in_maps


def assemble(results, S):
    """Gather per-core outputs into full (out, attn)."""
    NB = S // NBLK
    nbg = NBLK // GROUP
    attn = np.empty((B, H, S, S), np.float32)
    out = np.empty((B, S, D), np.float32)
    for c in range(NCORES):
        b, j = c // GROUP, c % GROUP
        attn[b, j * HPC : (j + 1) * HPC] = results[c]["attn"]
        y = results[c]["y"].reshape(NB, nbg, D)
        for n in range(NB):
            out[b, n * NBLK + j * nbg : n * NBLK + (j + 1) * nbg] = y[n]
    return out, attn


def kernel(q, k, v, W_q, b_q, W_k, b_k, W_v, b_v, W_o, b_o):
    from concourse import bass2jax

    S = int(np.asarray(q).shape[1])
    nc = _get_nc(S)
    in_maps = make_in_maps(q, k, v, W_q, b_q, W_k, b_k, W_v, b_v, W_o, b_o)
    results = bass2jax.run_bass_via_pjrt(nc, in_maps, n_cores=NCORES)
    return assemble(results, S)
